# revision 27
# baseline (speedup 1.0000x reference)
"""AnomalyAttention (causal multi-head attention) on 8 TRN2 NeuronCores.

Problem: B=2, C=4, H=8, L=1024, E=64  ->  64 independent heads.
Sharding: 8 heads per core (data parallel over B*C*H), no collectives.

Per-core pipeline (heads processed in pairs; head 2p uses PE row group 0:64,
head 2p+1 uses 64:128 so their QK^T matmuls run concurrently in the array):
  S^T[k, q] = sum_e K[k, e] Q[q, e]        (TensorE; lhsT = K^T chunk, rhs = Q^T)
  P^T = exp(0.125 * S^T)                   (ScalarE ACT, optionally VectorE
                                            Schraudolph for offloaded units)
  causal: upper blocks skipped, diagonal blocks * tri 0/1 (VectorE)
  [O | r][q, :] = sum_k P^T[k, q] * [V | 1][k, :]   (TensorE, PSUM accumulate)
  out[q, e] = O[q, e] / r[q]               (VectorE reciprocal + broadcast mult)

The causal S^T is packed into 9 [128,1024] psum units per head pair
(triple-buffered: 3 psum bufs x 2 banks), one exp op per unit, so QK^T of
unit u+2 overlaps exp of unit u and the whole body is ACT-paced.  AV for
q-blocks 0-3 of each head launches mid-pair (after units 4/5), filling the
PE while ACT drains the exp backlog.

Inputs are pre-transposed / bf16-cast / padded on the host as part of sharding.
Output DRAM layout [h, q%128, (q//128)*64 + e] for contiguous per-partition DMA.
"""

import os
import numpy as np
from ml_dtypes import bfloat16

import concourse.bass as bass
import concourse.tile as tile
from concourse import bacc, mybir
from concourse.bass_utils import run_bass_kernel_spmd

B, C, H, L, E = 2, 4, 8, 1024, 64
N_CORES = 8
HEADS = B * C * H              # 64
HPC = HEADS // N_CORES         # 8 heads per core
NPAIR = HPC // 2               # 4 head pairs per core
NCHUNK = L // 128              # 8 k-chunks of 128
NUNIT = 9                      # S^T units of [128, 1024] per pair
SCALE = 1.0 / 8.0

# ---------------------------------------------------------------------------
# Custom DVE op: corrected Schraudolph exp (optional ACT offload).
# Step 1 (stock tensor_scalar): i16 = int16(s * A + B); bits viewed as bf16
# give P0 = 2^e * (1+f) where e + f quantizes y = s*0.125*log2(e).
# Step 2 (this op): m = bits(P0) & 0xFF800000 -> 2^e (pure); n = ~m =
# -2^(2-e)*(1-2^-24); t = P0*n = -4(1+f).  q(t) is the minimax quadratic of
# 2^f/(1+f) over t in [-8,-4]; out = P0 * q(t) = 2^(e+f) to ~0.7%.
from concourse.dve_spec import (
    Spec as _Spec, Src0 as _Src0, C0 as _C0, C1 as _C1, C2 as _C2, C3 as _C3,
    AluOp as _AluOp, Bin as _Bin, _spill_c3_to_src1,
)
from concourse import dve_ops as _dve_ops
from concourse.dve_ops import DveOp as _DveOp

SCH_A = SCALE * 1.4426950408889634 * 128.0     # 23.083122...
SCH_B = 127.0 * 128.0                          # 16256.0
SCH_P0 = 1.43450618624325                      # q(t) = P2*t^2 + P1*t + P0
SCH_P1 = 0.16667197515057278
SCH_P2 = 0.01410901253114891


def _sch_ref(in0, in1, s0, s1, imm2):
    x = np.asarray(in0).astype(np.float32)
    bits = x.view(np.int32)
    m = (bits & np.int32(-8388608)).view(np.float32)
    n = (~m.view(np.int32)).view(np.float32)
    t = x * n
    q = (s1 * t + imm2) * t + np.asarray(in1).astype(np.float32)[:, :1]
    return (x * q).astype(np.float32)


_m = _Bin(_AluOp.BITWISE_AND, _Src0, _C0)
_n = _Bin(_AluOp.BITWISE_NOT, _m, _m)
_t = _Src0 * _n
_q = (_C1 * _t + _C2) * _t + _C3
SCH_CORR = _DveOp(
    "SCH_CORR",
    _Spec(body=_spill_c3_to_src1(_Src0 * _q), reference=_sch_ref),
    subdim=False,
    uops_sha={"v3": "0555131f0b12510f", "v4": "e336b2725d88d44b"},
)

# Fused normalize: out = in0 * approx_recip(in1) (NOT-seed + 1 Newton pass,
# ~0.17% max err on r in [0.5, 300]); replaces reciprocal + broadcast mult.
MR_C0 = -0.23549792
MR_C1 = 2.0017324


def _mr_ref(in0, in1, s0, s1, imm2):
    x1 = np.asarray(in1).astype(np.float32)
    nn = (~x1.view(np.int32)).view(np.float32)
    y0 = nn * np.float32(s0)
    y1 = y0 * (np.float32(s1) - x1 * y0)
    return (np.asarray(in0).astype(np.float32) * y1).astype(np.float32)


from concourse.dve_spec import Src1 as _Src1

_mn = _Bin(_AluOp.BITWISE_NOT, _Src1, _Src1)
_my0 = _mn * _C0
_my1 = _my0 * (_C1 - _Src1 * _my0)
MUL_RECIP = _DveOp(
    "MUL_RECIP",
    _Spec(body=_Src0 * _my1, reference=_mr_ref),
    subdim=False,
    uops_sha={"v3": "419098057196a8b3", "v4": "d769acdea7cb2ba5"},
)

for _op in (SCH_CORR, MUL_RECIP):
    if _op.name not in _dve_ops._SUB_OPCODE_FOR_NAME:
        _dve_ops.OPS.append(_op)
        _dve_ops.CUSTOM_DVE_SPECS[_op.name] = _op.spec
        _dve_ops._SUB_OPCODE_FOR_NAME[_op.name] = (
            max(_dve_ops._SUB_OPCODE_FOR_NAME.values()) + 1
        )


# ---------------------------------------------------------------------------
# Causal S^T unit packing: per pair, blocks (h, chunk i, q-block j>=i) laid
# chunk-major per head, heads interleaved across units so consecutive units'
# QK^T matmuls land on alternating PE row groups and run concurrently.
def _gen_units():
    streams = {}
    for h in (0, 1):
        streams[h] = [(h, i, j) for i in range(NCHUNK) for j in range(i, NCHUNK)]
    units = []
    for u in range(8):
        h = u % 2
        k = (u // 2) * 8
        units.append(streams[h][k:k + 8])
    units.append(streams[0][32:36] + streams[1][32:36])

    packs = []
    for blocks in units:
        runs = []
        for idx, (h, i, j) in enumerate(blocks):
            col = idx * 128
            if (runs and runs[-1][0] == h and runs[-1][1] == i
                    and runs[-1][3] == j
                    and runs[-1][4] // 512 == (col + 127) // 512):
                runs[-1][3] = j + 1
            else:
                runs.append([h, i, j, j + 1, col])
        diags = [idx * 128 for idx, (h, i, j) in enumerate(blocks) if i == j]
        packs.append((blocks, [tuple(r) for r in runs], diags))
    return packs


S_UNITS = _gen_units()
# after which unit index each (h_loc, half) AV unit may start
AV_GATES = {(0, 0): 4, (1, 0): 5, (0, 1): 8, (1, 1): 8}

LAST_RESULTS = None


class _Core:
    """Holds tiles + emission helpers for one SPMD program."""

    def __init__(self, nc, pools, qT_t, kT_t, vo_t, tri_t, msk_t, c3_t, mc0_t,
                 out, causal):
        self.nc = nc
        self.pools = pools
        self.qT_t, self.kT_t, self.vo_t = qT_t, kT_t, vo_t
        self.tri_t, self.msk_t, self.c3_t = tri_t, msk_t, c3_t
        self.mc0_t = mc0_t
        self.out = out
        self.causal = causal
        dve_u = os.environ.get("BASS_ATTN_DVE_UNITS", "1,6")
        self.dve_units = {int(x) for x in dve_u.split(",") if x != ""}
        # fused normalize is blocked by the single DVE PSUM read port
        # (in0 and the r broadcast would both come from PSUM)
        self.fused_norm = bool(int(os.environ.get("BASS_ATTN_FUSED_NORM", "0")))
        self.pmaps = {}
        self.av_po = {}

    def _exp(self, pt, ps, cols, on_dve):
        nc = self.nc
        i16 = mybir.dt.int16
        if on_dve:
            nc.vector.tensor_scalar(
                pt[:, 0:cols].bitcast(i16), ps[:, 0:cols], SCH_A, SCH_B,
                mybir.AluOpType.mult, mybir.AluOpType.add,
            )
            nc.vector._custom_dve(
                SCH_CORR, out=pt[:, 0:cols], in0=pt[:, 0:cols],
                in1=self.c3_t[:, :], s0=self.mc0_t[:, :], s1=SCH_P2,
                imm2=SCH_P1,
            )
        else:
            nc.scalar.activation(
                pt[:, 0:cols], ps[:, 0:cols],
                mybir.ActivationFunctionType.Exp, scale=SCALE,
            )

    def _mask_cols(self, pt, cols):
        """Multiply the 128-wide diag blocks at `cols` (a regular stride
        apart) by the lower-tri 0/1 mask in one strided VectorE op."""
        nc = self.nc
        ptap = pt[:, :]
        tap = self.tri_t[:, :]
        if len(cols) == 1:
            nc.vector.tensor_mul(pt[:, cols[0]:cols[0] + 128],
                                 pt[:, cols[0]:cols[0] + 128], self.tri_t)
            return
        step = cols[1] - cols[0]
        assert all(cols[k + 1] - cols[k] == step for k in range(len(cols) - 1))
        src = bass.AP(tensor=ptap.tensor, offset=ptap.offset + cols[0],
                      ap=[ptap.ap[0], [step, len(cols)], [1, 128]])
        trib = bass.AP(tensor=tap.tensor, offset=tap.offset,
                       ap=[tap.ap[0], [0, len(cols)], [1, 128]])
        nc.vector.tensor_mul(src, src, trib)

    def s_unit(self, p, u):
        """Emit QK^T + exp + mask for unit u of head pair p."""
        nc = self.nc
        bf = mybir.dt.bfloat16
        f32 = mybir.dt.float32
        psumS, pTpool = self.pools["psumS"], self.pools["pT"]
        for h_loc in (0, 1):
            self.pmaps.setdefault(2 * p + h_loc, {})

        blocks, runs, diags = S_UNITS[u]
        ps = psumS.tile([128, 1024], f32, tag="psS", name="psS")
        for (h_loc, i, j0, j1, col) in runs:
            rows = slice(64 * h_loc, 64 * h_loc + 64)
            w = (j1 - j0) * 128
            nc.tensor.matmul(
                ps[:, col:col + w],
                lhsT=self.kT_t[p][rows, 128 * i:128 * i + 128],
                rhs=self.qT_t[p][rows, 128 * j0:128 * j0 + w],
                start=True, stop=True,
            )
        pt = pTpool.tile([128, 1024], bf, tag="pt", name="pt")
        self._exp(pt, ps, 1024, u in self.dve_units)
        if diags:
            self._mask_cols(pt, diags)
        for idx, (h_loc, i, j) in enumerate(blocks):
            self.pmaps[2 * p + h_loc][(i, j)] = (pt, idx * 128)

    def s_chunk_noncausal(self, p, ci):
        """Non-causal fallback: full [128, 1024] S^T chunk per head."""
        nc = self.nc
        bf = mybir.dt.bfloat16
        f32 = mybir.dt.float32
        psumS, pTpool = self.pools["psumS"], self.pools["pT"]
        for h_loc in (0, 1):
            self.pmaps.setdefault(2 * p + h_loc, {})
        ps = [psumS.tile([128, 1024], f32, tag="psS", name="psS")
              for _ in range(2)]
        for s0 in range(0, L, 512):
            for h_loc, rows in ((0, slice(0, 64)), (1, slice(64, 128))):
                nc.tensor.matmul(
                    ps[h_loc][:, s0:s0 + 512],
                    lhsT=self.kT_t[p][rows, 128 * ci:128 * ci + 128],
                    rhs=self.qT_t[p][rows, s0:s0 + 512],
                    start=True, stop=True,
                )
        for h_loc in (0, 1):
            h = 2 * p + h_loc
            pt = pTpool.tile([128, 1024], bf, tag="pt", name="pt")
            nc.scalar.activation(
                pt, ps[h_loc], mybir.ActivationFunctionType.Exp, scale=SCALE,
            )
            nc.vector.tensor_mul(pt, pt, self.msk_t[ci])
            for j in range(NCHUNK):
                self.pmaps[h][(ci, j)] = (pt, 128 * j)

    def av_blocks(self, h, half, bis):
        """Emit AV matmuls for q-blocks {4*half+bi : bi in bis} of head h.
        All four blocks of a half accumulate in one single-bank psum tile
        [128, 260] held in self.av_po until av_norm drains it."""
        nc = self.nc
        f32 = mybir.dt.float32
        pmap = self.pmaps[h]
        key = (h, half)
        if key not in self.av_po:
            self.av_po[key] = self.pools["psumO"].tile(
                [128, 260], f32, tag="psO", name="psO")
        po = self.av_po[key]
        for bi in bis:
            j, coff = 4 * half + bi, 65 * bi
            ks = list(range(j + 1)) if self.causal else list(range(NCHUNK))
            for idx, i in enumerate(ks):
                pt, cs = pmap[(i, j)]
                nc.tensor.matmul(
                    po[:, coff:coff + 65],
                    lhsT=pt[:, cs:cs + 128],
                    rhs=self.vo_t[h][:, 65 * i:65 * i + 65],
                    start=(idx == 0), stop=(idx == len(ks) - 1),
                )

    def av_norm(self, h, half, obuf):
        """Normalize q-blocks of (h, half): one reciprocal + one broadcast
        multiply over the four 65-col psum regions."""
        nc = self.nc
        f32 = mybir.dt.float32
        rinvp = self.pools["rinvp"]
        po = self.av_po.pop((h, half))
        poap = po[:, :]
        o_in = bass.AP(tensor=poap.tensor, offset=poap.offset,
                       ap=[poap.ap[0], [65, 4], [1, 64]])
        oap = obuf[:, :]
        o_out = bass.AP(tensor=oap.tensor, offset=oap.offset + 256 * half,
                        ap=[oap.ap[0], [64, 4], [1, 64]])
        if self.fused_norm:
            # one DVE op: out = O * approx_recip(r), r broadcast along e
            r_bc = bass.AP(tensor=poap.tensor, offset=poap.offset + 64,
                           ap=[poap.ap[0], [65, 4], [0, 64]])
            nc.vector._custom_dve(
                MUL_RECIP, out=o_out, in0=o_in, in1=r_bc,
                s0=MR_C0, s1=MR_C1,
            )
        else:
            # reciprocal of the four denominators (cols 64+65*bi)
            rinv4 = rinvp.tile([128, 4], f32, tag="rinv", name="rinv")
            rsrc = bass.AP(tensor=poap.tensor, offset=poap.offset + 64,
                           ap=[poap.ap[0], [65, 4]])
            nc.vector.reciprocal(rinv4, rsrc)
            rap = rinv4[:, :]
            r_in = bass.AP(tensor=rap.tensor, offset=rap.offset,
                           ap=[rap.ap[0], [1, 4], [0, 64]])
            nc.vector.tensor_mul(o_out, o_in, r_in)


def _build(causal: bool):
    nc = bacc.Bacc("TRN2", target_bir_lowering=False, debug=False,
                   num_devices=N_CORES)
    bf = mybir.dt.bfloat16
    f32 = mybir.dt.float32

    qT = nc.dram_tensor("qT", [NPAIR, 128, L], bf, kind="ExternalInput").ap()
    kT = nc.dram_tensor("kT", [NPAIR, 128, L], bf, kind="ExternalInput").ap()
    vo = nc.dram_tensor("vo", [HPC, 128, NCHUNK * 65], bf, kind="ExternalInput").ap()
    tri = nc.dram_tensor("tri", [128, 128], bf, kind="ExternalInput").ap()
    if not causal:
        msk = nc.dram_tensor("msk", [NCHUNK, 128, L], bf, kind="ExternalInput").ap()
    out = nc.dram_tensor("out", [HPC, 128, 512], bf, kind="ExternalOutput").ap()

    with tile.TileContext(nc) as tc:
        with (
            tc.tile_pool(name="consts", bufs=1) as consts,
            tc.tile_pool(name="pT", bufs=20 if causal else 32) as pTpool,
            tc.tile_pool(name="psumS", bufs=3 if causal else 2,
                         space="PSUM") as psumS,
            tc.tile_pool(name="psumO", bufs=2, space="PSUM") as psumO,
            tc.tile_pool(name="outsb", bufs=2) as outsb,
            tc.tile_pool(name="rinvp", bufs=4) as rinvp,
        ):
            pools = dict(psumS=psumS, psumO=psumO, pT=pTpool, rinvp=rinvp)
            # pair-0 K/Q land first on two separate DMA queues so QK^T
            # starts as early as possible; everything else follows
            qT_t = [consts.tile([128, L], bf, tag=f"qT{p}", name=f"qTt{p}")
                    for p in range(NPAIR)]
            kT_t = [consts.tile([128, L], bf, tag=f"kT{p}", name=f"kTt{p}")
                    for p in range(NPAIR)]
            vo_t = [consts.tile([128, NCHUNK * 65], bf, tag=f"vo{h}",
                                name=f"vot{h}")
                    for h in range(HPC)]
            tri_t = consts.tile([128, 128], bf, tag="tri")
            nc.sync.dma_start(out=kT_t[0], in_=kT[0])
            nc.scalar.dma_start(out=qT_t[0], in_=qT[0])
            nc.gpsimd.dma_start(out=tri_t, in_=tri)
            nc.gpsimd.dma_start(out=vo_t[0], in_=vo[0])
            nc.gpsimd.dma_start(out=vo_t[1], in_=vo[1])
            nc.sync.dma_start(out=kT_t[1], in_=kT[1])
            nc.scalar.dma_start(out=qT_t[1], in_=qT[1])
            for p in (2, 3):
                nc.sync.dma_start(out=kT_t[p], in_=kT[p])
                nc.sync.dma_start(out=qT_t[p], in_=qT[p])
            for h in range(2, HPC):
                nc.gpsimd.dma_start(out=vo_t[h], in_=vo[h])
            msk_t = []
            if not causal:
                for c in range(NCHUNK):
                    t = consts.tile([128, L], bf, tag=f"msk{c}", name=f"mskt{c}")
                    nc.gpsimd.dma_start(out=t, in_=msk[c])
                    msk_t.append(t)

            # per-partition latch constant for SCH_CORR's third coefficient,
            # and the sign+exponent mask (-inf bits; walrus can't carry an
            # inf immediate through its json, so it rides a memset tile)
            c3_t = consts.tile([128, 1], f32, tag="c3")
            nc.vector.memset(c3_t, SCH_P0)
            mc0_t = consts.tile([128, 1], f32, tag="mc0")
            nc.vector.memset(mc0_t, float("-inf"))
            # warm the ACT exp table early so the first real exp is fast
            warm = consts.tile([128, 8], f32, tag="warm")
            nc.vector.memset(warm, 0.0)
            warm2 = consts.tile([128, 8], f32, tag="warm2")
            nc.scalar.activation(warm2, warm, mybir.ActivationFunctionType.Exp)
            # HAM warm-up: dummy matmuls on zeros fill the ~7-11us engine/DMA
            # startup dead zone so the real matmuls start at 2.4 GHz
            wsrc = consts.tile([128, 260], bf, tag="wsrc")
            nc.vector.memset(wsrc, 0.0)
            wps = psumO.tile([128, 260], f32, tag="psO", name="wps")

            def pe_warm(n):
                for _ in range(n):
                    nc.tensor.matmul(wps, lhsT=wsrc[:, 0:128], rhs=wsrc,
                                     start=True, stop=True)
            pe_warm(20)

            core = _Core(nc, pools, qT_t, kT_t, vo_t, tri_t, msk_t, c3_t,
                         mc0_t, out, causal)

            # spread output slabs across the three DMA rings so the write-
            # back drains in parallel with the sync ring's input traffic
            out_rings = [nc.sync]
            ring_ctr = [0]

            def norm_out(h, half, obufs):
                ob = obufs[h]
                core.av_norm(h, half, ob)
                ring = out_rings[ring_ctr[0] % len(out_rings)]
                ring_ctr[0] += 1
                ring.dma_start(
                    out=out[h][:, 256 * half:256 * half + 256],
                    in_=ob[:, 256 * half:256 * half + 256])

            def do_av(h, half, obufs):
                core.av_blocks(h, half, (0, 1, 2, 3))
                norm_out(h, half, obufs)

            if causal:
                # pipeline: units u0..u8 per pair; AV half0 of each head
                # launches mid-pair; AV half1 of pair p is split into
                # 2-block pieces threaded between the first units of pair
                # p+1 so the next pair's QK^T (and thus ACT) is never
                # starved behind a long AV burst
                obufs = {}
                for p in range(NPAIR):
                    for h in (2 * p, 2 * p + 1):
                        obufs[h] = outsb.tile([128, 512], bf,
                                              tag=f"ob{h % 4}", name=f"ob{h}")
                    hA, hB = 2 * p, 2 * p + 1
                    pA, pB = 2 * (p - 1), 2 * (p - 1) + 1
                    for u in range(NUNIT):
                        core.s_unit(p, u)
                        if p > 0:
                            if u == 0:
                                core.av_blocks(pA, 1, (0, 1))
                            elif u == 1:
                                core.av_blocks(pA, 1, (2, 3))
                                norm_out(pA, 1, obufs)
                            elif u == 2:
                                core.av_blocks(pB, 1, (0, 1))
                            elif u == 3:
                                core.av_blocks(pB, 1, (2, 3))
                                norm_out(pB, 1, obufs)
                                obufs.pop(pA)
                                obufs.pop(pB)
                        if u == 4:
                            core.av_blocks(hA, 0, (0, 1))
                        elif u == 5:
                            core.av_blocks(hA, 0, (2, 3))
                            norm_out(hA, 0, obufs)
                            core.av_blocks(hB, 0, (0, 1))
                        elif u == 6:
                            core.av_blocks(hB, 0, (2, 3))
                            norm_out(hB, 0, obufs)
                p = NPAIR - 1
                do_av(2 * p, 1, obufs)
                do_av(2 * p + 1, 1, obufs)
            else:
                obufs = {}
                for p in range(NPAIR):
                    for h in (2 * p, 2 * p + 1):
                        obufs[h] = outsb.tile([128, 512], bf,
                                              tag=f"ob{h % 4}", name=f"ob{h}")
                    for ci in range(NCHUNK):
                        core.s_chunk_noncausal(p, ci)
                    for half in (0, 1):
                        for h in (2 * p, 2 * p + 1):
                            do_av(h, half, obufs)
    nc.compile()
    return nc


_CACHE = {}


def _get_nc(causal: bool):
    if causal not in _CACHE:
        _CACHE[causal] = _build(causal)
    return _CACHE[causal]


def kernel(queries, keys, values, attn_mask):
    global LAST_RESULTS
    q = np.asarray(queries).reshape(HEADS, L, E)
    k = np.asarray(keys).reshape(HEADS, L, E)
    v = np.asarray(values).reshape(HEADS, L, E)
    mask = np.asarray(attn_mask).reshape(L, L)
    causal = bool(np.array_equal(mask, np.triu(np.ones((L, L), bool), k=1)))

    nc = _get_nc(causal)

    tri = np.triu(np.ones((128, 128), np.float32), k=0).astype(bfloat16)
    if not causal:
        # reference mask is [query, key]; S^T layout needs [key, query]
        m01 = np.where(mask, 0.0, 1.0).astype(np.float32).T
        msk = np.ascontiguousarray(m01).reshape(NCHUNK, 128, L).astype(bfloat16)

    in_maps = []
    for c in range(N_CORES):
        hs = slice(c * HPC, (c + 1) * HPC)
        qTm = np.ascontiguousarray(
            q[hs].transpose(0, 2, 1)).astype(bfloat16).reshape(NPAIR, 128, L)
        kTm = np.ascontiguousarray(
            k[hs].transpose(0, 2, 1)).astype(bfloat16).reshape(NPAIR, 128, L)
        vh = v[hs].astype(np.float32)
        vcat = np.concatenate(
            [vh, np.ones((HPC, L, 1), np.float32)], axis=2)  # [8, L, 65]
        vom = np.ascontiguousarray(
            vcat.reshape(HPC, NCHUNK, 128, 65).transpose(0, 2, 1, 3)
        ).astype(bfloat16).reshape(HPC, 128, NCHUNK * 65)
        im = {"qT": qTm, "kT": kTm, "vo": vom, "tri": tri}
        if not causal:
            im["msk"] = msk
        in_maps.append(im)

    trace = bool(os.environ.get("BASS_ATTN_TRACE"))
    res = run_bass_kernel_spmd(nc, in_maps, core_ids=list(range(N_CORES)),
                               trace=trace)
    LAST_RESULTS = res
    # out[c]: [HPC, 128, 512] = [h, p, j*64+e]; q = 128*j + p
    outs = np.stack([res.results[c]["out"] for c in range(N_CORES)])
    outs = outs.reshape(N_CORES, HPC, 128, NCHUNK, E).transpose(0, 1, 3, 2, 4)
    return np.ascontiguousarray(
        outs.reshape(B, C, H, L, E)).astype(np.float32)


# revision 29
# speedup vs baseline: 1.2108x; 1.2108x over previous
"""AnomalyAttention (causal multi-head attention) on 8 TRN2 NeuronCores.

Problem: B=2, C=4, H=8, L=1024, E=64  ->  64 independent heads.
Sharding: 8 heads per core (data parallel over B*C*H), no collectives.

Per-core pipeline (heads processed in pairs; head 2p uses PE row group 0:64,
head 2p+1 uses 64:128 so their QK^T matmuls run concurrently in the array):
  S^T[k, q] = sum_e K[k, e] Q[q, e]        (TensorE; lhsT = K^T chunk, rhs = Q^T)
  P^T = exp(0.125 * S^T)                   (ScalarE ACT, optionally VectorE
                                            Schraudolph for offloaded units)
  causal: upper blocks skipped, diagonal blocks * tri 0/1 (VectorE)
  [O | r][q, :] = sum_k P^T[k, q] * [V | 1][k, :]   (TensorE, PSUM accumulate)
  out[q, e] = O[q, e] / r[q]               (VectorE reciprocal + broadcast mult)

The causal S^T is packed into 9 [128,1024] psum units per head pair
(triple-buffered: 3 psum bufs x 2 banks), one exp op per unit, so QK^T of
unit u+2 overlaps exp of unit u and the whole body is ACT-paced.  AV for
q-blocks 0-3 of each head launches mid-pair (after units 4/5), filling the
PE while ACT drains the exp backlog.

Inputs are pre-transposed / bf16-cast / padded on the host as part of sharding.
Output DRAM layout [h, q%128, (q//128)*64 + e] for contiguous per-partition DMA.
"""

import os
import numpy as np
from ml_dtypes import bfloat16

import concourse.bass as bass
import concourse.tile as tile
from concourse import bacc, mybir
from concourse.bass_utils import run_bass_kernel_spmd

B, C, H, L, E = 2, 4, 8, 1024, 64
N_CORES = 8
HEADS = B * C * H              # 64
HPC = HEADS // N_CORES         # 8 heads per core
NPAIR = HPC // 2               # 4 head pairs per core
NCHUNK = L // 128              # 8 k-chunks of 128
NUNIT = 9                      # S^T units of [128, 1024] per pair
SCALE = 1.0 / 8.0

# ---------------------------------------------------------------------------
# Custom DVE op: corrected Schraudolph exp (optional ACT offload).
# Step 1 (stock tensor_scalar): i16 = int16(s * A + B); bits viewed as bf16
# give P0 = 2^e * (1+f) where e + f quantizes y = s*0.125*log2(e).
# Step 2 (this op): m = bits(P0) & 0xFF800000 -> 2^e (pure); n = ~m =
# -2^(2-e)*(1-2^-24); t = P0*n = -4(1+f).  q(t) is the minimax quadratic of
# 2^f/(1+f) over t in [-8,-4]; out = P0 * q(t) = 2^(e+f) to ~0.7%.
from concourse.dve_spec import (
    Spec as _Spec, Src0 as _Src0, C0 as _C0, C1 as _C1, C2 as _C2, C3 as _C3,
    AluOp as _AluOp, Bin as _Bin, _spill_c3_to_src1,
)
from concourse import dve_ops as _dve_ops
from concourse.dve_ops import DveOp as _DveOp

SCH_A = SCALE * 1.4426950408889634 * 128.0     # 23.083122...
SCH_B = 127.0 * 128.0                          # 16256.0
SCH_P0 = 1.43450618624325                      # q(t) = P2*t^2 + P1*t + P0
SCH_P1 = 0.16667197515057278
SCH_P2 = 0.01410901253114891


def _sch_ref(in0, in1, s0, s1, imm2):
    x = np.asarray(in0).astype(np.float32)
    bits = x.view(np.int32)
    m = (bits & np.int32(-8388608)).view(np.float32)
    n = (~m.view(np.int32)).view(np.float32)
    t = x * n
    q = (s1 * t + imm2) * t + np.asarray(in1).astype(np.float32)[:, :1]
    return (x * q).astype(np.float32)


_m = _Bin(_AluOp.BITWISE_AND, _Src0, _C0)
_n = _Bin(_AluOp.BITWISE_NOT, _m, _m)
_t = _Src0 * _n
_q = (_C1 * _t + _C2) * _t + _C3
SCH_CORR = _DveOp(
    "SCH_CORR",
    _Spec(body=_spill_c3_to_src1(_Src0 * _q), reference=_sch_ref),
    subdim=False,
    uops_sha={"v3": "0555131f0b12510f", "v4": "e336b2725d88d44b"},
)

# Fused normalize: out = in0 * approx_recip(in1) (NOT-seed + 1 Newton pass,
# ~0.17% max err on r in [0.5, 300]); replaces reciprocal + broadcast mult.
MR_C0 = -0.23549792
MR_C1 = 2.0017324


def _mr_ref(in0, in1, s0, s1, imm2):
    x1 = np.asarray(in1).astype(np.float32)
    nn = (~x1.view(np.int32)).view(np.float32)
    y0 = nn * np.float32(s0)
    y1 = y0 * (np.float32(s1) - x1 * y0)
    return (np.asarray(in0).astype(np.float32) * y1).astype(np.float32)


from concourse.dve_spec import Src1 as _Src1

_mn = _Bin(_AluOp.BITWISE_NOT, _Src1, _Src1)
_my0 = _mn * _C0
_my1 = _my0 * (_C1 - _Src1 * _my0)
MUL_RECIP = _DveOp(
    "MUL_RECIP",
    _Spec(body=_Src0 * _my1, reference=_mr_ref),
    subdim=False,
    uops_sha={"v3": "419098057196a8b3", "v4": "d769acdea7cb2ba5"},
)

for _op in (SCH_CORR, MUL_RECIP):
    if _op.name not in _dve_ops._SUB_OPCODE_FOR_NAME:
        _dve_ops.OPS.append(_op)
        _dve_ops.CUSTOM_DVE_SPECS[_op.name] = _op.spec
        _dve_ops._SUB_OPCODE_FOR_NAME[_op.name] = (
            max(_dve_ops._SUB_OPCODE_FOR_NAME.values()) + 1
        )


# ---------------------------------------------------------------------------
# Causal S^T unit packing: per pair, blocks (h, chunk i, q-block j>=i) laid
# chunk-major per head, heads interleaved across units so consecutive units'
# QK^T matmuls land on alternating PE row groups and run concurrently.
def _gen_units():
    streams = {}
    for h in (0, 1):
        streams[h] = [(h, i, j) for i in range(NCHUNK) for j in range(i, NCHUNK)]
    units = []
    for u in range(8):
        h = u % 2
        k = (u // 2) * 8
        units.append(streams[h][k:k + 8])
    units.append(streams[0][32:36] + streams[1][32:36])

    packs = []
    for blocks in units:
        runs = []
        for idx, (h, i, j) in enumerate(blocks):
            col = idx * 128
            if (runs and runs[-1][0] == h and runs[-1][1] == i
                    and runs[-1][3] == j
                    and runs[-1][4] // 512 == (col + 127) // 512):
                runs[-1][3] = j + 1
            else:
                runs.append([h, i, j, j + 1, col])
        diags = [idx * 128 for idx, (h, i, j) in enumerate(blocks) if i == j]
        packs.append((blocks, [tuple(r) for r in runs], diags))
    return packs


S_UNITS = _gen_units()
# after which unit index each (h_loc, half) AV unit may start
AV_GATES = {(0, 0): 4, (1, 0): 5, (0, 1): 8, (1, 1): 8}

LAST_RESULTS = None


class _Core:
    """Holds tiles + emission helpers for one SPMD program."""

    def __init__(self, nc, pools, qT_t, kT_t, vo_t, tri_t, msk_t, c3_t, mc0_t,
                 out, causal):
        self.nc = nc
        self.pools = pools
        self.qT_t, self.kT_t, self.vo_t = qT_t, kT_t, vo_t
        self.tri_t, self.msk_t, self.c3_t = tri_t, msk_t, c3_t
        self.mc0_t = mc0_t
        self.out = out
        self.causal = causal
        dve_u = os.environ.get("BASS_ATTN_DVE_UNITS", "1,6")
        self.dve_units = {int(x) for x in dve_u.split(",") if x != ""}
        # fused normalize is blocked by the single DVE PSUM read port
        # (in0 and the r broadcast would both come from PSUM)
        self.fused_norm = bool(int(os.environ.get("BASS_ATTN_FUSED_NORM", "0")))
        self.pmaps = {}
        self.av_po = {}

    def _exp(self, pt, ps, cols, on_dve):
        nc = self.nc
        i16 = mybir.dt.int16
        if on_dve:
            nc.vector.tensor_scalar(
                pt[:, 0:cols].bitcast(i16), ps[:, 0:cols], SCH_A, SCH_B,
                mybir.AluOpType.mult, mybir.AluOpType.add,
            )
            nc.vector._custom_dve(
                SCH_CORR, out=pt[:, 0:cols], in0=pt[:, 0:cols],
                in1=self.c3_t[:, :], s0=self.mc0_t[:, :], s1=SCH_P2,
                imm2=SCH_P1,
            )
        else:
            nc.scalar.activation(
                pt[:, 0:cols], ps[:, 0:cols],
                mybir.ActivationFunctionType.Exp, scale=SCALE,
            )

    def _mask_cols(self, pt, cols):
        """Multiply the 128-wide diag blocks at `cols` (a regular stride
        apart) by the lower-tri 0/1 mask in one strided VectorE op."""
        nc = self.nc
        ptap = pt[:, :]
        tap = self.tri_t[:, :]
        if len(cols) == 1:
            nc.vector.tensor_mul(pt[:, cols[0]:cols[0] + 128],
                                 pt[:, cols[0]:cols[0] + 128], self.tri_t)
            return
        step = cols[1] - cols[0]
        assert all(cols[k + 1] - cols[k] == step for k in range(len(cols) - 1))
        src = bass.AP(tensor=ptap.tensor, offset=ptap.offset + cols[0],
                      ap=[ptap.ap[0], [step, len(cols)], [1, 128]])
        trib = bass.AP(tensor=tap.tensor, offset=tap.offset,
                       ap=[tap.ap[0], [0, len(cols)], [1, 128]])
        nc.vector.tensor_mul(src, src, trib)

    def s_unit(self, p, u):
        """Emit QK^T + exp + mask for unit u of head pair p."""
        nc = self.nc
        bf = mybir.dt.bfloat16
        f32 = mybir.dt.float32
        psumS, pTpool = self.pools["psumS"], self.pools["pT"]
        for h_loc in (0, 1):
            self.pmaps.setdefault(2 * p + h_loc, {})

        blocks, runs, diags = S_UNITS[u]
        ps = psumS.tile([128, 1024], f32, tag="psS", name="psS")
        for (h_loc, i, j0, j1, col) in runs:
            rows = slice(64 * h_loc, 64 * h_loc + 64)
            w = (j1 - j0) * 128
            nc.tensor.matmul(
                ps[:, col:col + w],
                lhsT=self.kT_t[p][rows, 128 * i:128 * i + 128],
                rhs=self.qT_t[p][rows, 128 * j0:128 * j0 + w],
                start=True, stop=True,
            )
        pt = pTpool.tile([128, 1024], bf, tag="pt", name="pt")
        # the last pair's unit 6 gates its AV half-1 tail directly; keep it
        # on the fast ACT path so the kernel tail is not stretched by the
        # slower VectorE exp chain
        on_dve = u in self.dve_units and not (p == NPAIR - 1 and u >= 6)
        self._exp(pt, ps, 1024, on_dve)
        if diags:
            self._mask_cols(pt, diags)
        for idx, (h_loc, i, j) in enumerate(blocks):
            self.pmaps[2 * p + h_loc][(i, j)] = (pt, idx * 128)

    def s_chunk_noncausal(self, p, ci):
        """Non-causal fallback: full [128, 1024] S^T chunk per head."""
        nc = self.nc
        bf = mybir.dt.bfloat16
        f32 = mybir.dt.float32
        psumS, pTpool = self.pools["psumS"], self.pools["pT"]
        for h_loc in (0, 1):
            self.pmaps.setdefault(2 * p + h_loc, {})
        ps = [psumS.tile([128, 1024], f32, tag="psS", name="psS")
              for _ in range(2)]
        for s0 in range(0, L, 512):
            for h_loc, rows in ((0, slice(0, 64)), (1, slice(64, 128))):
                nc.tensor.matmul(
                    ps[h_loc][:, s0:s0 + 512],
                    lhsT=self.kT_t[p][rows, 128 * ci:128 * ci + 128],
                    rhs=self.qT_t[p][rows, s0:s0 + 512],
                    start=True, stop=True,
                )
        for h_loc in (0, 1):
            h = 2 * p + h_loc
            pt = pTpool.tile([128, 1024], bf, tag="pt", name="pt")
            nc.scalar.activation(
                pt, ps[h_loc], mybir.ActivationFunctionType.Exp, scale=SCALE,
            )
            nc.vector.tensor_mul(pt, pt, self.msk_t[ci])
            for j in range(NCHUNK):
                self.pmaps[h][(ci, j)] = (pt, 128 * j)

    def av_blocks(self, h, half, bis):
        """Emit AV matmuls for q-blocks {4*half+bi : bi in bis} of head h.
        All four blocks of a half accumulate in one single-bank psum tile
        [128, 260] held in self.av_po until av_norm drains it."""
        nc = self.nc
        f32 = mybir.dt.float32
        pmap = self.pmaps[h]
        key = (h, half)
        if key not in self.av_po:
            self.av_po[key] = self.pools["psumO"].tile(
                [128, 260], f32, tag="psO", name="psO")
        po = self.av_po[key]
        for bi in bis:
            j, coff = 4 * half + bi, 65 * bi
            ks = list(range(j + 1)) if self.causal else list(range(NCHUNK))
            for idx, i in enumerate(ks):
                pt, cs = pmap[(i, j)]
                nc.tensor.matmul(
                    po[:, coff:coff + 65],
                    lhsT=pt[:, cs:cs + 128],
                    rhs=self.vo_t[h][:, 65 * i:65 * i + 65],
                    start=(idx == 0), stop=(idx == len(ks) - 1),
                )

    def av_norm(self, h, half, obuf):
        """Normalize q-blocks of (h, half): one reciprocal + one broadcast
        multiply over the four 65-col psum regions."""
        nc = self.nc
        f32 = mybir.dt.float32
        rinvp = self.pools["rinvp"]
        po = self.av_po.pop((h, half))
        poap = po[:, :]
        o_in = bass.AP(tensor=poap.tensor, offset=poap.offset,
                       ap=[poap.ap[0], [65, 4], [1, 64]])
        oap = obuf[:, :]
        o_out = bass.AP(tensor=oap.tensor, offset=oap.offset + 256 * half,
                        ap=[oap.ap[0], [64, 4], [1, 64]])
        if self.fused_norm:
            # one DVE op: out = O * approx_recip(r), r broadcast along e
            r_bc = bass.AP(tensor=poap.tensor, offset=poap.offset + 64,
                           ap=[poap.ap[0], [65, 4], [0, 64]])
            nc.vector._custom_dve(
                MUL_RECIP, out=o_out, in0=o_in, in1=r_bc,
                s0=MR_C0, s1=MR_C1,
            )
        else:
            # reciprocal of the four denominators (cols 64+65*bi)
            rinv4 = rinvp.tile([128, 4], f32, tag="rinv", name="rinv")
            rsrc = bass.AP(tensor=poap.tensor, offset=poap.offset + 64,
                           ap=[poap.ap[0], [65, 4]])
            nc.vector.reciprocal(rinv4, rsrc)
            rap = rinv4[:, :]
            r_in = bass.AP(tensor=rap.tensor, offset=rap.offset,
                           ap=[rap.ap[0], [1, 4], [0, 64]])
            nc.vector.tensor_mul(o_out, o_in, r_in)


def _build(causal: bool):
    nc = bacc.Bacc("TRN2", target_bir_lowering=False, debug=False,
                   num_devices=N_CORES)
    bf = mybir.dt.bfloat16
    f32 = mybir.dt.float32

    qT = nc.dram_tensor("qT", [NPAIR, 128, L], bf, kind="ExternalInput").ap()
    kT = nc.dram_tensor("kT", [NPAIR, 128, L], bf, kind="ExternalInput").ap()
    vo = nc.dram_tensor("vo", [HPC, 128, NCHUNK * 65], bf, kind="ExternalInput").ap()
    tri = nc.dram_tensor("tri", [128, 128], bf, kind="ExternalInput").ap()
    if not causal:
        msk = nc.dram_tensor("msk", [NCHUNK, 128, L], bf, kind="ExternalInput").ap()
    out = nc.dram_tensor("out", [HPC, 128, 512], bf, kind="ExternalOutput").ap()

    with tile.TileContext(nc) as tc:
        with (
            tc.tile_pool(name="consts", bufs=1) as consts,
            tc.tile_pool(name="pT", bufs=20 if causal else 32) as pTpool,
            tc.tile_pool(name="psumS", bufs=3 if causal else 2,
                         space="PSUM") as psumS,
            tc.tile_pool(name="psumO", bufs=2, space="PSUM") as psumO,
            tc.tile_pool(name="outsb", bufs=2) as outsb,
            tc.tile_pool(name="rinvp", bufs=4) as rinvp,
        ):
            pools = dict(psumS=psumS, psumO=psumO, pT=pTpool, rinvp=rinvp)
            # pair-0 K/Q land first on two separate DMA queues so QK^T
            # starts as early as possible; everything else follows
            qT_t = [consts.tile([128, L], bf, tag=f"qT{p}", name=f"qTt{p}")
                    for p in range(NPAIR)]
            kT_t = [consts.tile([128, L], bf, tag=f"kT{p}", name=f"kTt{p}")
                    for p in range(NPAIR)]
            vo_t = [consts.tile([128, NCHUNK * 65], bf, tag=f"vo{h}",
                                name=f"vot{h}")
                    for h in range(HPC)]
            tri_t = consts.tile([128, 128], bf, tag="tri")
            nc.sync.dma_start(out=kT_t[0], in_=kT[0])
            nc.scalar.dma_start(out=qT_t[0], in_=qT[0])
            nc.gpsimd.dma_start(out=tri_t, in_=tri)
            nc.gpsimd.dma_start(out=vo_t[0], in_=vo[0])
            nc.gpsimd.dma_start(out=vo_t[1], in_=vo[1])
            nc.sync.dma_start(out=kT_t[1], in_=kT[1])
            nc.scalar.dma_start(out=qT_t[1], in_=qT[1])
            for p in (2, 3):
                nc.sync.dma_start(out=kT_t[p], in_=kT[p])
                nc.sync.dma_start(out=qT_t[p], in_=qT[p])
            for h in range(2, HPC):
                nc.gpsimd.dma_start(out=vo_t[h], in_=vo[h])
            msk_t = []
            if not causal:
                for c in range(NCHUNK):
                    t = consts.tile([128, L], bf, tag=f"msk{c}", name=f"mskt{c}")
                    nc.gpsimd.dma_start(out=t, in_=msk[c])
                    msk_t.append(t)

            # per-partition latch constant for SCH_CORR's third coefficient,
            # and the sign+exponent mask (-inf bits; walrus can't carry an
            # inf immediate through its json, so it rides a memset tile)
            c3_t = consts.tile([128, 1], f32, tag="c3")
            nc.vector.memset(c3_t, SCH_P0)
            mc0_t = consts.tile([128, 1], f32, tag="mc0")
            nc.vector.memset(mc0_t, float("-inf"))
            # warm the ACT exp table early so the first real exp is fast
            warm = consts.tile([128, 8], f32, tag="warm")
            nc.vector.memset(warm, 0.0)
            warm2 = consts.tile([128, 8], f32, tag="warm2")
            nc.scalar.activation(warm2, warm, mybir.ActivationFunctionType.Exp)
            # HAM warm-up: dummy matmuls on zeros fill the ~7-11us engine/DMA
            # startup dead zone so the real matmuls start at 2.4 GHz
            wsrc = consts.tile([128, 260], bf, tag="wsrc")
            nc.vector.memset(wsrc, 0.0)
            wps = psumO.tile([128, 260], f32, tag="psO", name="wps")

            def pe_warm(n):
                for _ in range(n):
                    nc.tensor.matmul(wps, lhsT=wsrc[:, 0:128], rhs=wsrc,
                                     start=True, stop=True)
            # ~14 x 400ns of dummy matmuls covers the ~4.5us DMA dead zone
            # and sustains PE busy long enough for the HAM warm grant without
            # overrunning into the first real QK^T work
            pe_warm(14)

            core = _Core(nc, pools, qT_t, kT_t, vo_t, tri_t, msk_t, c3_t,
                         mc0_t, out, causal)

            # spread output slabs across the three DMA rings so the write-
            # back drains in parallel with the sync ring's input traffic
            out_rings = [nc.sync]
            ring_ctr = [0]

            def norm_out(h, half, obufs):
                ob = obufs[h]
                core.av_norm(h, half, ob)
                ring = out_rings[ring_ctr[0] % len(out_rings)]
                ring_ctr[0] += 1
                ring.dma_start(
                    out=out[h][:, 256 * half:256 * half + 256],
                    in_=ob[:, 256 * half:256 * half + 256])

            def do_av(h, half, obufs):
                core.av_blocks(h, half, (0, 1, 2, 3))
                norm_out(h, half, obufs)

            if causal:
                # pipeline: units u0..u8 per pair; AV half0 of each head
                # launches mid-pair; AV half1 of pair p is split into
                # 2-block pieces threaded between the first units of pair
                # p+1 so the next pair's QK^T (and thus ACT) is never
                # starved behind a long AV burst
                obufs = {}
                for p in range(NPAIR):
                    for h in (2 * p, 2 * p + 1):
                        obufs[h] = outsb.tile([128, 512], bf,
                                              tag=f"ob{h % 4}", name=f"ob{h}")
                    hA, hB = 2 * p, 2 * p + 1
                    pA, pB = 2 * (p - 1), 2 * (p - 1) + 1
                    for u in range(NUNIT):
                        core.s_unit(p, u)
                        if p > 0:
                            if u == 0:
                                core.av_blocks(pA, 1, (0, 1))
                            elif u == 1:
                                core.av_blocks(pA, 1, (2, 3))
                                norm_out(pA, 1, obufs)
                            elif u == 2:
                                core.av_blocks(pB, 1, (0, 1))
                            elif u == 3:
                                core.av_blocks(pB, 1, (2, 3))
                                norm_out(pB, 1, obufs)
                                obufs.pop(pA)
                                obufs.pop(pB)
                        if u == 4:
                            core.av_blocks(hA, 0, (0, 1))
                        elif u == 5:
                            core.av_blocks(hA, 0, (2, 3))
                            norm_out(hA, 0, obufs)
                            core.av_blocks(hB, 0, (0, 1))
                        elif u == 6:
                            core.av_blocks(hB, 0, (2, 3))
                            norm_out(hB, 0, obufs)
                p = NPAIR - 1
                do_av(2 * p, 1, obufs)
                do_av(2 * p + 1, 1, obufs)
            else:
                obufs = {}
                for p in range(NPAIR):
                    for h in (2 * p, 2 * p + 1):
                        obufs[h] = outsb.tile([128, 512], bf,
                                              tag=f"ob{h % 4}", name=f"ob{h}")
                    for ci in range(NCHUNK):
                        core.s_chunk_noncausal(p, ci)
                    for half in (0, 1):
                        for h in (2 * p, 2 * p + 1):
                            do_av(h, half, obufs)
    nc.compile()
    return nc


_CACHE = {}


def _get_nc(causal: bool):
    if causal not in _CACHE:
        _CACHE[causal] = _build(causal)
    return _CACHE[causal]


def kernel(queries, keys, values, attn_mask):
    global LAST_RESULTS
    q = np.asarray(queries).reshape(HEADS, L, E)
    k = np.asarray(keys).reshape(HEADS, L, E)
    v = np.asarray(values).reshape(HEADS, L, E)
    mask = np.asarray(attn_mask).reshape(L, L)
    causal = bool(np.array_equal(mask, np.triu(np.ones((L, L), bool), k=1)))

    nc = _get_nc(causal)

    tri = np.triu(np.ones((128, 128), np.float32), k=0).astype(bfloat16)
    if not causal:
        # reference mask is [query, key]; S^T layout needs [key, query]
        m01 = np.where(mask, 0.0, 1.0).astype(np.float32).T
        msk = np.ascontiguousarray(m01).reshape(NCHUNK, 128, L).astype(bfloat16)

    in_maps = []
    for c in range(N_CORES):
        hs = slice(c * HPC, (c + 1) * HPC)
        qTm = np.ascontiguousarray(
            q[hs].transpose(0, 2, 1)).astype(bfloat16).reshape(NPAIR, 128, L)
        kTm = np.ascontiguousarray(
            k[hs].transpose(0, 2, 1)).astype(bfloat16).reshape(NPAIR, 128, L)
        vh = v[hs].astype(np.float32)
        vcat = np.concatenate(
            [vh, np.ones((HPC, L, 1), np.float32)], axis=2)  # [8, L, 65]
        vom = np.ascontiguousarray(
            vcat.reshape(HPC, NCHUNK, 128, 65).transpose(0, 2, 1, 3)
        ).astype(bfloat16).reshape(HPC, 128, NCHUNK * 65)
        im = {"qT": qTm, "kT": kTm, "vo": vom, "tri": tri}
        if not causal:
            im["msk"] = msk
        in_maps.append(im)

    trace = bool(os.environ.get("BASS_ATTN_TRACE"))
    res = run_bass_kernel_spmd(nc, in_maps, core_ids=list(range(N_CORES)),
                               trace=trace)
    LAST_RESULTS = res
    # out[c]: [HPC, 128, 512] = [h, p, j*64+e]; q = 128*j + p
    outs = np.stack([res.results[c]["out"] for c in range(N_CORES)])
    outs = outs.reshape(N_CORES, HPC, 128, NCHUNK, E).transpose(0, 1, 3, 2, 4)
    return np.ascontiguousarray(
        outs.reshape(B, C, H, L, E)).astype(np.float32)
